# revision 9
# baseline (speedup 1.0000x reference)
"""AlphaStock Trainium2 kernel (8 NeuronCores, SPMD), v2.

Model: per-asset LSTM(T=64, H=128) + temporal attention pooling (HA), then
cross-asset attention (CAAN) over M=512 assets per batch element.

Sharding: the B*M=1024 sequences are split 128-per-core for the LSTM/HA
stage (launch A). The tiny CAAN stage runs as a second launch (B) sharded
by query rows (cores 0-3 -> batch 0, cores 4-7 -> batch 1), with the
gathered per-asset representations re-broadcast by the host between the
two launches.

v2 LSTM cell ("all-tanh" reformulation): one Tanh activation over all
4 gates replaces sigmoid(i,f)+tanh(g)+sigmoid(o), using
  sigmoid(x) = (tanh(x/2)+1)/2
with the 1/2 pre-scales folded into the weights host-side. The DVE
combines gates with fused scalar_tensor_tensor ops:
  T = tanh([a_o/2, a_i/2, a_f/2, a_g])            (one ACT, 4H wide)
  prodt = (T[i|f] + 1) * [T_g | c]  = [2*si*tg | 2*sf*c]
  s = prodt_i + prodt_f = 2*c'
  c' = 0.5*s (off critical path), th = tanh(0.5*s) (ACT scale)
  Hh = (T_o + 1) * th = 2*h
The h-state is carried as 2h; compensating 0.5 factors are folded into
W_hh, w1, w2 and a final host-side 0.5 on the gathered stock_rep.

Layouts (launch A, per core, n = 128 local sequences):
  - gates PSUM tile [128 units, 512] per step, gate order (o, i, f, g)
  - h stored transposed: rep[h, (n, t)]  (free index = n*64 + t)
  - x pre-transposed on host: xT[f(+1 ones row), t*128+n]; the ones row
    carries the (pre-scaled) b_ih+b_hh through the x-matmul.
"""

import numpy as np

B, M, T, F, H = 2, 512, 64, 16, 128
N_CORES = 8
NPC = (B * M) // N_CORES  # sequences per core = 128
G4 = 4 * H  # 512
GH = 64      # sequences per pipeline group (2 groups of 64 per core)

_CACHE = {}


def _gate_perm():
    # torch gate order (i, f, g, o) -> kernel order (o, i, f, g)
    idx = np.arange(4 * H).reshape(4, H)
    return np.concatenate([idx[3], idx[0], idx[1], idx[2]])


def _build_launch_a():
    import concourse.bacc as bacc
    import concourse.tile as tile
    import concourse.bass as bass
    from concourse import mybir
    from contextlib import ExitStack

    f32 = mybir.dt.float32
    f16 = mybir.dt.float16
    AF = mybir.ActivationFunctionType
    ALU = mybir.AluOpType

    nc = bacc.Bacc("TRN2", target_bir_lowering=False, debug=False,
                   num_devices=N_CORES)

    xT = nc.dram_tensor("xT", [F + 1, T * NPC], f16, kind="ExternalInput").ap()
    wih = nc.dram_tensor("wih", [F + 1, G4], f16, kind="ExternalInput").ap()
    whh = nc.dram_tensor("whh", [H, G4], f16, kind="ExternalInput").ap()
    w1T = nc.dram_tensor("w1T", [H, H], f16, kind="ExternalInput").ap()
    w2T = nc.dram_tensor("w2T", [H, H], f16, kind="ExternalInput").ap()
    b12 = nc.dram_tensor("b12", [H, 1], f32, kind="ExternalInput").ap()
    waT = nc.dram_tensor("waT", [H, H], f16, kind="ExternalInput").ap()
    stock = nc.dram_tensor("stock", [H, NPC], f32, kind="ExternalOutput").ap()

    with tile.TileContext(nc) as tc, ExitStack() as ctx:
        big = ctx.enter_context(tc.tile_pool(name="big", bufs=1))
        state = ctx.enter_context(tc.tile_pool(name="state", bufs=1))
        work = ctx.enter_context(tc.tile_pool(name="work", bufs=4))
        psum = ctx.enter_context(tc.tile_pool(name="psum", bufs=3, space="PSUM"))
        psumw = ctx.enter_context(tc.tile_pool(name="psumw", bufs=1, space="PSUM"))

        # ---- resident tensors
        xsb = big.tile([F + 1, T * NPC], f16, tag="xsb")
        rep = big.tile([H, NPC * T], f16, tag="rep")     # rep[h, t*128+n] = 2h

        wih_sb = state.tile([F + 1, G4], f16, tag="wih")
        whh_sb = state.tile([H, G4], f16, tag="whh")
        w1T_sb = state.tile([H, H], f16, tag="w1T")
        w2T_sb = state.tile([H, H], f16, tag="w2T")
        b12_sb = state.tile([H, 1], f32, tag="b12")
        waT_sb = state.tile([H, H], f16, tag="waT")
        # Per sequence-group tanh outputs [to, ti, tf, tg] in cols 0:4*GH
        # and cell state c in 4*GH:5*GH (GH = 64 sequences per group).
        tstX = state.tile([H, 5 * GH], f16, tag="tstX")
        tstY = state.tile([H, 5 * GH], f16, tag="tstY")
        # DMAs spread across per-engine rings (each ring feeds its own DMA
        # engine at ~27 GB/s; a single ring serializes the whole 450KB).
        # x chunk 0 and wih go first on separate rings so the recurrence
        # can start as early as possible; whh is only needed at t=1.
        XCH = T * NPC // 8
        nc.sync.dma_start(out=xsb[:, 0:XCH], in_=xT[:, 0:XCH])
        nc.gpsimd.dma_start(out=wih_sb, in_=wih)
        nc.scalar.dma_start(out=whh_sb, in_=whh)
        for j in range(1, 8):
            eng = (nc.sync, nc.gpsimd, nc.scalar)[j % 3]
            eng.dma_start(out=xsb[:, j * XCH:(j + 1) * XCH],
                          in_=xT[:, j * XCH:(j + 1) * XCH])
        nc.scalar.dma_start(out=w1T_sb, in_=w1T)
        nc.scalar.dma_start(out=w2T_sb, in_=w2T)
        nc.scalar.dma_start(out=b12_sb, in_=b12)
        nc.scalar.dma_start(out=waT_sb, in_=waT)
        nc.vector.memset(tstX[:, 4 * GH:5 * GH], 0.0)  # c0 = 0
        nc.vector.memset(tstY[:, 4 * GH:5 * GH], 0.0)

        # ---- LSTM over T steps.
        # Engine streams execute in program order: x-matmuls are emitted
        # XAHEAD steps early so the PE runs them while it waits for h_{t-1},
        # and only the 4 W_hh matmuls sit on the recurrence critical path.
        XAHEAD = 2
        ps_tiles = {}

        def emit_x(t):
            ps = psum.tile([H, G4], f32, tag="gates")
            ps_tiles[t] = ps
            for g in range(4):
                gs = slice(g * H, (g + 1) * H)
                # g==0 start=True zeroes the whole 2KB bank (zero region);
                # gates 1-3 land on pending-zero bytes and overwrite.
                nc.tensor.matmul(ps[:, gs], lhsT=wih_sb[:, gs],
                                 rhs=xsb[:, t * NPC:(t + 1) * NPC],
                                 start=(g == 0), stop=True,
                                 skip_group_check=(g != 0))

        def cell_group(t, tst, g0, tags):
            # one LSTM cell update for sequences [g0, g0+GH) of step t
            ps = ps_tiles[t]
            gin = ps.rearrange("p (g n) -> p g n", n=NPC)[:, :, g0:g0 + GH]
            nc.scalar.activation(
                tst[:, 0:4 * GH].rearrange("p (g n) -> p g n", n=GH),
                gin, AF.Tanh)
            prodt = work.tile([H, 2 * GH], f16, tag="pr" + tags)
            nc.vector.scalar_tensor_tensor(
                prodt, tst[:, GH:3 * GH], 1.0, tst[:, 3 * GH:5 * GH],
                ALU.add, ALU.mult)
            s2 = work.tile([H, GH], f16, tag="s2" + tags)
            nc.vector.tensor_add(s2, prodt[:, 0:GH], prodt[:, GH:2 * GH])
            th = work.tile([H, GH], f16, tag="th" + tags)
            nc.scalar.activation(th, s2, AF.Tanh, scale=0.5)
            # Hh = (T_o + 1) * th = 2*h, written straight into rep (t-major)
            h_cur = rep[:, t * NPC + g0:t * NPC + g0 + GH]
            nc.vector.scalar_tensor_tensor(
                h_cur, tst[:, 0:GH], 1.0, th, ALU.add, ALU.mult)
            # c' = s/2 back into the c slot, off the critical path
            nc.vector.tensor_scalar_mul(tst[:, 4 * GH:5 * GH], s2, 0.5)

        for t in range(XAHEAD):
            emit_x(t)
        for t in range(T):
            ps = ps_tiles[t]
            if t > 0:
                hp = rep[:, (t - 1) * NPC:t * NPC]
                for g0 in (0, GH):
                    for g in range(4):
                        nc.tensor.matmul(ps[:, g * H + g0:g * H + g0 + GH],
                                         lhsT=whh_sb[:, g * H:(g + 1) * H],
                                         rhs=hp[:, g0:g0 + GH],
                                         start=False, stop=True,
                                         skip_group_check=True)
            if t + XAHEAD < T:
                emit_x(t + XAHEAD)
            # two independent sequence groups pipeline through ACT/DVE/PE:
            # group Y runs roughly half a step behind group X.
            cell_group(t, tstX, 0, "x")
            cell_group(t, tstY, GH, "y")
            ps_tiles.pop(t)

        # ---- HA attention pooling (t-major rep), 4 groups of 4 chunks.
        # Each chunk covers 4 time steps x all 128 sequences; the softmax
        # over t spans all groups, so per-group partial sums are combined
        # at the end.
        CH = 512             # free elems per chunk
        NCH = NPC * T // CH  # 16 chunks
        TCH = CH // NPC      # 4 time steps per chunk
        GRP = 4              # chunks per group
        GT = GRP * TCH       # 16 time steps per group
        GW = GRP * CH        # 2048 free elems per group
        hl = rep[:, (T - 1) * NPC:T * NPC]  # 2*h_63, fp16 [H, NPC]
        stock_sb = state.tile([H, NPC], f32, tag="stock_sb")
        pssum = state.tile([H, 4, NPC], f32, tag="pssum")
        pstku = state.tile([H, 4, NPC], f32, tag="pstku")
        ssum = state.tile([H, NPC], f32, tag="ssum")
        stku = state.tile([H, NPC], f32, tag="stku")
        rr = state.tile([H, NPC], f32, tag="rr")

        def tree_reduce_t(dst, src, w):
            # dst [H, NPC] f32, src [H, w, NPC] fp16 view: sum over axis 1
            # via halving tensor_adds (2x DVE mode) instead of a 1x-mode
            # tensor_reduce.
            cur = src
            while w > 2:
                nxt_t = work.tile([H, NPC * (w // 2)], f16, tag=f"tr{w}")
                nxt = nxt_t.rearrange("p (t n) -> p t n", n=NPC)
                nc.vector.tensor_add(nxt, cur[:, 0:w // 2, :],
                                     cur[:, w // 2:w, :])
                cur = nxt
                w //= 2
            nc.vector.tensor_add(dst, cur[:, 0, :], cur[:, 1, :])

        for grp in range(NCH // GRP):
            for lc in range(GRP):
                ch = grp * GRP + lc
                cs = slice(ch * CH, (ch + 1) * CH)
                aps = psum.tile([H, CH], f32, tag="gates")
                nc.tensor.matmul(aps, lhsT=w1T_sb, rhs=rep[:, cs],
                                 start=True, stop=False)
                # a2 contribution: h_last broadcast over t via 0-stride rhs
                hl_b = bass.AP(tensor=hl.tensor, offset=hl.offset,
                               ap=[hl.ap[0], [0, TCH], *hl.ap[1:]])
                nc.tensor.matmul(aps.rearrange("p (t n) -> p t n", n=NPC),
                                 lhsT=w2T_sb, rhs=hl_b,
                                 start=False, stop=True)
                z = work.tile([H, CH], f16, tag="z")
                nc.scalar.activation(z, aps, AF.Tanh, bias=b12_sb)
                if lc == 0:
                    wps = psumw.tile([H, GW], f32, tag="wps")
                nc.tensor.matmul(wps[:, lc * CH:(lc + 1) * CH], lhsT=waT_sb,
                                 rhs=z, start=True, stop=True)
            gsl = slice(grp * GW, (grp + 1) * GW)
            # wps rows are replicated across all 128 partitions, so the whole
            # softmax + weighted sum runs full-lane with no cross-partition
            # moves: exp, per-n partial sums, unnormalized partial stock.
            eU = work.tile([H, GW], f16, tag="eU")
            nc.scalar.activation(eU, wps, AF.Exp)
            eU3 = eU.rearrange("p (t n) -> p t n", n=NPC)
            tree_reduce_t(pssum[:, grp, :], eU3, GT)
            nc.vector.tensor_mul(eU, rep[:, gsl], eU)
            tree_reduce_t(pstku[:, grp, :], eU3, GT)
        # combine group partials: softmax denominator and weighted sum
        cmb = state.tile([H, 2 * NPC], f32, tag="cmb")
        cmb3 = cmb.rearrange("p (t n) -> p t n", n=NPC)
        nc.vector.tensor_add(cmb3, pssum[:, 0:2, :], pssum[:, 2:4, :])
        nc.vector.tensor_add(ssum, cmb3[:, 0, :], cmb3[:, 1, :])
        nc.vector.reciprocal(rr, ssum)
        nc.vector.tensor_add(cmb3, pstku[:, 0:2, :], pstku[:, 2:4, :])
        nc.vector.tensor_add(stku, cmb3[:, 0, :], cmb3[:, 1, :])
        nc.vector.tensor_mul(stock_sb, stku, rr)
        nc.sync.dma_start(out=stock, in_=stock_sb)

    nc.compile()
    return nc


def _build_launch_b():
    import concourse.bacc as bacc
    import concourse.tile as tile
    from concourse import mybir
    from contextlib import ExitStack

    f32 = mybir.dt.float32
    f16 = mybir.dt.float16
    AF = mybir.ActivationFunctionType

    nc = bacc.Bacc("TRN2", target_bir_lowering=False, debug=False,
                   num_devices=N_CORES)

    xrT = nc.dram_tensor("xrT", [H, M], f16, kind="ExternalInput").ap()
    xqT = nc.dram_tensor("xqT", [H, NPC], f16, kind="ExternalInput").ap()
    # packed: [wqT | wkT | wvT | eye | wwT-col]
    wpk = nc.dram_tensor("wpk", [H, 4 * H + 1], f16, kind="ExternalInput").ap()
    # packed: [bq | bk | cst-broadcast-row]
    bpk = nc.dram_tensor("bpk", [H, 3], f32, kind="ExternalInput").ap()
    scores = nc.dram_tensor("scores", [1, NPC], f32, kind="ExternalOutput").ap()

    with tile.TileContext(nc) as tc, ExitStack() as ctx:
        pool = ctx.enter_context(tc.tile_pool(name="sb", bufs=1))
        psum = ctx.enter_context(tc.tile_pool(name="ps", bufs=2, space="PSUM"))
        psum1 = ctx.enter_context(tc.tile_pool(name="ps1", bufs=1, space="PSUM"))

        xrT_sb = pool.tile([H, M], f16, tag="xrT")
        xqT_sb = pool.tile([H, NPC], f16, tag="xqT")
        wpk_sb = pool.tile([H, 4 * H + 1], f16, tag="wpk")
        bpk_sb = pool.tile([H, 3], f32, tag="bpk")
        nc.sync.dma_start(out=xrT_sb, in_=xrT)
        nc.sync.dma_start(out=xqT_sb, in_=xqT)
        nc.sync.dma_start(out=wpk_sb, in_=wpk)
        nc.sync.dma_start(out=bpk_sb, in_=bpk)
        wqT_sb = wpk_sb[:, 0:H]
        wkT_sb = wpk_sb[:, H:2 * H]
        wvT_sb = wpk_sb[:, 2 * H:3 * H]
        eye_sb = wpk_sb[:, 3 * H:4 * H]
        wwT_sb = wpk_sb[:, 4 * H:4 * H + 1]
        bq_sb = bpk_sb[:, 0:1]
        bk_sb = bpk_sb[:, 1:2]
        cst_sb = bpk_sb[:, 2:3]

        # q/k projections (transposed layout [h', *])
        qps = psum.tile([H, NPC], f32, tag="ps")
        nc.tensor.matmul(qps, lhsT=wqT_sb, rhs=xqT_sb, start=True, stop=True)
        qsb = pool.tile([H, NPC], f16, tag="qsb")
        nc.scalar.activation(qsb, qps, AF.Identity, bias=bq_sb)

        kps = psum1.tile([H, M], f32, tag="kps")
        nc.tensor.matmul(kps, lhsT=wkT_sb, rhs=xrT_sb, start=True, stop=True)
        ksb = pool.tile([H, M], f16, tag="ksb")
        nc.scalar.activation(ksb, kps, AF.Identity, bias=bk_sb)

        # v in [k, h'] layout (no bias: beta rows sum to 1, folded into cst)
        vsb = pool.tile([H, 4, H], f16, tag="vsb")
        for j in range(4):
            vps = psum.tile([H, H], f32, tag="ps")
            nc.tensor.matmul(vps, lhsT=xrT_sb[:, j * H:(j + 1) * H],
                             rhs=wvT_sb, start=True, stop=True)
            nc.scalar.activation(vsb[:, j, :], vps, AF.Copy)

        # S = q^T k / sqrt(H); e = exp
        sps = psum1.tile([NPC, M], f32, tag="sps")
        nc.tensor.matmul(sps, lhsT=qsb, rhs=ksb, start=True, stop=True)
        esb = pool.tile([NPC, M], f16, tag="esb")
        nc.scalar.activation(esb, sps, AF.Exp, scale=float(1.0 / np.sqrt(H)))
        ssum = pool.tile([NPC, 1], f32, tag="ssum")
        nc.vector.tensor_reduce(ssum, esb, mybir.AxisListType.X,
                                mybir.AluOpType.add)
        rr = pool.tile([NPC, 1], f32, tag="rr")
        nc.vector.reciprocal(rr, ssum)
        nc.vector.tensor_scalar_mul(esb, esb, rr)

        # transpose e chunks -> eT [k, q], then attnT = sum_j v_j @ eT_j
        eT = pool.tile([H, 4, NPC], f16, tag="eT")
        for j in range(4):
            tps = psum.tile([H, NPC], f16, tag="tp")
            nc.tensor.transpose(tps, esb[:, j * H:(j + 1) * H], eye_sb)
            nc.vector.tensor_copy(eT[:, j, :], tps)
        aps = psum1.tile([H, NPC], f32, tag="aps")
        for j in range(4):
            nc.tensor.matmul(aps, lhsT=vsb[:, j, :], rhs=eT[:, j, :],
                             start=(j == 0), stop=(j == 3))
        attn = pool.tile([H, NPC], f16, tag="attn")
        nc.scalar.activation(attn, aps, AF.Copy)

        scps = psum1.tile([1, NPC], f32, tag="scps")
        nc.tensor.matmul(scps, lhsT=wwT_sb, rhs=attn, start=True, stop=True)
        ssb = pool.tile([1, NPC], f32, tag="ssb")
        nc.scalar.activation(ssb, scps, AF.Identity, bias=cst_sb[0:1, :])
        nc.sync.dma_start(out=scores, in_=ssb)

    nc.compile()
    return nc


def _prep_inputs_a(inputs):
    perm = _gate_perm()
    W_ih = np.asarray(inputs["W_ih"])[perm].astype(np.float64)   # [512, 16]
    W_hh = np.asarray(inputs["W_hh"])[perm].astype(np.float64)   # [512, 128]
    bias = (np.asarray(inputs["b_ih"]) + np.asarray(inputs["b_hh"]))[perm]
    bias = bias.astype(np.float64)
    # all-tanh trick: o,i,f pre-activations scaled by 1/2; W_hh gets an
    # extra global 1/2 because the streamed h-state is 2h.
    colscale = np.concatenate([np.full(3 * H, 0.5), np.ones(H)])
    W_ih *= colscale[:, None]
    bias *= colscale
    W_hh *= 0.5 * colscale[:, None]
    wih = np.concatenate([W_ih.T, bias[None, :]], axis=0)  # [17, 512]
    whh = np.ascontiguousarray(W_hh.T)               # [128, 512]
    # rep holds 2h: fold 1/2 into w1 and w2
    w1T = np.ascontiguousarray(np.asarray(inputs["w1"]).T) * 0.5
    w2T = np.ascontiguousarray(np.asarray(inputs["w2"]).T) * 0.5
    b12 = (np.asarray(inputs["b1"]) + np.asarray(inputs["b2"]))[:, None]
    waT = np.repeat(np.asarray(inputs["wa"]).T, H, axis=1)  # [128, 128] replicated

    x = np.asarray(inputs["x"]).reshape(B * M, T, F)
    shared = dict(wih=np.ascontiguousarray(wih).astype(np.float16),
                  whh=whh.astype(np.float16),
                  w1T=w1T.astype(np.float16), w2T=w2T.astype(np.float16),
                  b12=np.ascontiguousarray(b12, np.float32),
                  waT=waT.astype(np.float16))
    in_maps = []
    for c in range(N_CORES):
        xc = x[c * NPC:(c + 1) * NPC]                # [128, 64, 16]
        xTc = np.empty((F + 1, T * NPC), np.float16)
        xTc[:F] = xc.transpose(2, 1, 0).reshape(F, T * NPC)  # [f, t*128+n]
        xTc[F] = 1.0
        in_maps.append(dict(xT=np.ascontiguousarray(xTc), **shared))
    return in_maps


def _prep_inputs_b(inputs, xr):
    # xr: [B, M, H] gathered stock_rep
    wqT = np.ascontiguousarray(np.asarray(inputs["wq"]).T).astype(np.float16)
    wkT = np.ascontiguousarray(np.asarray(inputs["wk"]).T).astype(np.float16)
    wvT = np.ascontiguousarray(np.asarray(inputs["wv"]).T).astype(np.float16)
    bq = np.ascontiguousarray(np.asarray(inputs["bq"])[:, None], np.float32)
    bk = np.ascontiguousarray(np.asarray(inputs["bk"])[:, None], np.float32)
    ww = np.asarray(inputs["ww"])                     # [1, H]
    bv = np.asarray(inputs["bv"])                     # [H]
    bw = np.asarray(inputs["bw"])                     # [1]
    wwT = np.ascontiguousarray(ww.T).astype(np.float16)
    cst = float(ww[0] @ bv + bw[0])
    eye = np.eye(H, dtype=np.float16)
    wpk = np.concatenate([wqT, wkT, wvT, eye, wwT], axis=1)
    bpk = np.concatenate([bq, bk, np.full((H, 1), cst, np.float32)], axis=1)
    wpk = np.ascontiguousarray(wpk)
    bpk = np.ascontiguousarray(bpk)

    in_maps = []
    for c in range(N_CORES):
        b, qc = c // 4, c % 4
        xrT = np.ascontiguousarray(xr[b].T).astype(np.float16)   # [H, M]
        xqT = np.ascontiguousarray(xrT[:, qc * NPC:(qc + 1) * NPC])
        in_maps.append(dict(xrT=xrT, xqT=xqT, wpk=wpk, bpk=bpk))
    return in_maps


def _get_programs():
    if "a" not in _CACHE:
        _CACHE["a"] = _build_launch_a()
    if "b" not in _CACHE:
        _CACHE["b"] = _build_launch_b()
    return _CACHE["a"], _CACHE["b"]


def _gather_xr(results_a):
    xr = np.empty((B, M, H), np.float32)
    for c in range(N_CORES):
        st = results_a[c]["stock"]                   # [H, NPC], holds 2x
        n0 = c * NPC
        b, m0 = divmod(n0, M)
        xr[b, m0:m0 + NPC] = 0.5 * st.T
    return xr


def _assemble_scores(results_b):
    out = np.empty((B, M), np.float32)
    for c in range(N_CORES):
        b, qc = c // 4, c % 4
        out[b, qc * NPC:(qc + 1) * NPC] = results_b[c]["scores"][0]
    return out


def kernel(**inputs):
    from concourse.bass_utils import run_bass_kernel_spmd

    nca, ncb = _get_programs()
    in_maps_a = _prep_inputs_a(inputs)
    res_a = run_bass_kernel_spmd(nca, in_maps_a, core_ids=list(range(N_CORES)))
    xr = _gather_xr(res_a.results)
    in_maps_b = _prep_inputs_b(inputs, xr)
    res_b = run_bass_kernel_spmd(ncb, in_maps_b, core_ids=list(range(N_CORES)))
    return _assemble_scores(res_b.results)
